# revision 12
# baseline (speedup 1.0000x reference)
"""DualHOILoss Trainium2 kernel (8 NeuronCores, pure data parallel over batch).

Math (per batch b, point p, object o):
    u = basis_p + delta_p,  w_o = o - m_b
    d2[p,o] = |u/s - w|^2 = u.(-2w/s) + |u|^2/s^2 + |w|^2
computed as ONE f32r matmul with K=5 rows: lhsT rows [ux,uy,uz,|u|^2,1],
rhs rows [-2wx/s,-2wy/s,-2wz/s,1/s^2,|w|^2] -> PSUM holds the COMPLETE d2
(no per-point correction). Tiles processed in pairs sharing one K=10 lhsT
slice (rows 0:5 even tile, 5:10 odd tile; rhs has two zero-padded variants).

Per pair: PE writes 390 vert cols/tile into psA (2 banks) and 388 into psB;
ACT drains psB pair -> SBUF c2 (INF-padded to 390); DVE tensor_tensor_scan
(min,min) folds psA against c2 into junk; the running-min tails are read
directly by ACT Exp via a strided AP (contacts). Choir path: host gathers
the selected anchor per point into wselT2 rows (same 5-row semantics);
Pool multiplies uT*wselT2 (bf16 out); tiny bf16 matmuls against a 0/1
column pair reduce the 10 rows -> d2sel [NT,128] in PSUM; DVE clamps,
ACT sqrts, Pool diffs/squares, PE ones-matmul colsums -> 4 scalars.

Host prep is layout-only + O(B*P) packing (transposes, anchor gather,
|u|^2): all O(B*P*V) work runs on device. lhsT/rhs are DMA'd as f32 and
rounded to f32r on-device by Pool copies (birverifier requirement).
Point tiling keeps the SBUF-natural map p = 32q + tau.
"""

import numpy as np

B, P, A, V = 16, 4096, 32, 778
NCORES = 8
BPC = B // NCORES      # batches per core
NT = P // 128          # 32 point tiles per batch
NPAIR = NT // 2        # 16 tile pairs per batch
N1 = 390               # vert cols scanned straight from PSUM (per tile)
N2 = V - N1            # vert cols drained to SBUF (388)
W = N1                 # scan width (c2 is INF-padded N2 -> W)
INF = 3.0e38

_CACHE = {}


def _build_program():
    import concourse.bacc as bacc
    import concourse.mybir as mybir
    from concourse import tile

    f32 = mybir.dt.float32
    f32r = mybir.dt.float32r
    bf16 = mybir.dt.bfloat16
    AF = mybir.ActivationFunctionType
    ALU = mybir.AluOpType
    AX = mybir.AxisListType

    nc = bacc.Bacc(None, target_bir_lowering=False)

    # ---- DRAM inputs (host-packed) ----
    # uT: [10, NPAIR*128] rows [ux,uy,uz,|u|^2,1] even tile / odd tile
    uT_d = nc.dram_tensor("uT", [BPC, 10, NPAIR * 128], f32, kind="ExternalInput")
    # wsel: same layout/semantics, selected-anchor rows per point
    ws_d = nc.dram_tensor("ws", [BPC, 10, NPAIR * 128], f32, kind="ExternalInput")
    # rhs: [10, 2*V]: cols 0:V variant-even (rows 0:5 data), V:2V variant-odd
    rhs_d = nc.dram_tensor("rhs", [BPC, 10, 2 * V], f32, kind="ExternalInput")
    # hc: hand contacts point-major [128, NT]; adT: anchor-dist targets [NT, 128]
    hc_d = nc.dram_tensor("hc", [BPC, 128, NT], f32, kind="ExternalInput")
    adT_d = nc.dram_tensor("adT", [BPC, NT, 128], f32, kind="ExternalInput")
    out_d = nc.dram_tensor("partials", [1, 2], f32, kind="ExternalOutput")

    # per-pair selector matrices: selmats[k, 32*kp + tau] = 1 iff
    # (k<5 and tau==2kp) or (k>=5 and tau==2kp+1); accumulated matmuls
    # build d2selT [NT, 128] in one PSUM bank.
    selmats = np.zeros((10, 32 * NPAIR), np.float32)
    for kp in range(NPAIR):
        selmats[0:5, 32 * kp + 2 * kp] = 1.0
        selmats[5:10, 32 * kp + 2 * kp + 1] = 1.0
    selm_d = nc.inline_tensor(selmats, "selm")
    cst = np.zeros((128, 4), np.float32)
    cst[:, 0] = 1.0                      # ones column (bf16 lhsT for colsum)
    cst_d = nc.inline_tensor(cst, "cst")

    with tile.TileContext(nc) as tc:
        with (
            tc.tile_pool(name="sb", bufs=1) as sb,            # persistent
            tc.tile_pool(name="sbb", bufs=2) as sbb,          # per-batch
            tc.tile_pool(name="sbj", bufs=2) as sbj,          # junk per-batch
            tc.tile_pool(name="sbc", bufs=2) as sbc,          # c2 ping-pong
            tc.tile_pool(name="sbt", bufs=4) as sbt,          # small tail bufs
            tc.tile_pool(name="psA", bufs=2, space="PSUM") as psA,   # 4 banks
            tc.tile_pool(name="psB", bufs=1, space="PSUM") as psB,   # 2 banks
            tc.tile_pool(name="psS", bufs=2, space="PSUM") as psS,   # 2 banks
        ):
            # constants
            cstf = sb.tile([128, 4], f32, tag="cstf")
            nc.sync.dma_start(cstf[:], cst_d[:])
            cstb = sb.tile([128, 4], bf16, tag="cstb")
            nc.vector.tensor_copy(cstb[:], cstf[:])
            ones128b = cstb[:, 0:1]
            ones32b = cstb[0:NT, 0:1]
            selmf = sb.tile([10, 32 * NPAIR], f32, tag="selmf")
            nc.sync.dma_start(selmf[:], selm_d[:])
            selmb = sb.tile([10, 32 * NPAIR], bf16, tag="selmb")
            nc.vector.tensor_copy(selmb[:], selmf[:])

            # c2 drain buffers with INF pads at cols N2:W per half
            c2bufs = []
            for i in range(2):
                c2 = sbc.tile([128, 2 * W], f32, tag=f"c2_{i}")
                nc.vector.memset(c2[:, N2:W], INF)
                nc.vector.memset(c2[:, W + N2:], INF)
                c2bufs.append(c2)

            # final loss accumulators in SBUF [1, 4*128]:
            # cols 0:256 choir (b0,b1), 256:512 contact (b0,b1)
            acc = sb.tile([1, 4 * 128], f32, tag="acc")
            nc.vector.memset(acc[:], 0.0)

            for b in range(BPC):
                # ---------------- per-batch loads + f32r rounding ----------
                uT_s = sbb.tile([10, NPAIR * 128], f32, tag="uT_s")
                nc.sync.dma_start(uT_s[:], uT_d[b])
                rhs_s = sbb.tile([10, 2 * V], f32, tag="rhs_s")
                nc.sync.dma_start(rhs_s[:], rhs_d[b])
                ws = sbb.tile([10, NPAIR * 128], f32, tag="ws")
                nc.sync.dma_start(ws[:], ws_d[b])
                hc = sbb.tile([128, NT], f32, tag="hc")
                nc.sync.dma_start(hc[:], hc_d[b])
                adT = sbb.tile([NT, 128], f32, tag="adT")
                nc.sync.dma_start(adT[:], adT_d[b])

                uT = sbb.tile([10, NPAIR * 128], f32r, tag="uT")
                nc.gpsimd.tensor_copy(uT[:], uT_s[:])
                rhs = sbb.tile([10, 2 * V], f32r, tag="rhs")
                nc.gpsimd.tensor_copy(rhs[:], rhs_s[:])

                # choir: prod = uT * wsel (bf16 out feeds the sel matmuls)
                prod = sbb.tile([10, NPAIR * 128], bf16, tag="prod")
                nc.gpsimd.tensor_tensor(
                    prod[:], uT[:].bitcast(f32), ws[:], op=ALU.mult)

                junk = sbj.tile([128, W * NT], f32, tag="junk")
                d2selT = psS.tile([NT, 512], f32, tag="pss")

                # ---------------- per-pair main loop ----------------
                for kp in range(NPAIR):
                    lhsT = uT[:, 128 * kp: 128 * (kp + 1)]
                    ptA = psA.tile([128, 1024], f32, tag="ptA")
                    ptB = psB.tile([128, 1024], f32, tag="ptB")
                    c2 = c2bufs[kp % 2]
                    for j in range(2):
                        rv = rhs[:, V * j: V * j + V]
                        nc.tensor.matmul(ptA[:, 512 * j: 512 * j + N1],
                                         lhsT, rv[:, 0:N1],
                                         start=True, stop=True)
                        nc.tensor.matmul(ptB[:, 512 * j: 512 * j + N2],
                                         lhsT, rv[:, N1:V],
                                         start=True, stop=True)
                    # selmm: accumulate this pair's d2sel rows into [NT, 128]
                    nc.tensor.matmul(d2selT[:, 0:128],
                                     selmb[:, 32 * kp: 32 * (kp + 1)],
                                     prod[:, 128 * kp: 128 * (kp + 1)],
                                     start=(kp == 0), stop=(kp == NPAIR - 1))
                    # drain psB pair -> c2 (cols 0:N2 / W:W+N2)
                    nc.scalar.activation(
                        c2[:].rearrange("p (j w) -> p j w", j=2)[:, :, 0:N2],
                        ptB[:].rearrange("p (j w) -> p j w", j=2)[:, :, 0:N2],
                        AF.Copy)
                    # min-min scans
                    for j in range(2):
                        t = 2 * kp + j
                        nc.vector.tensor_tensor_scan(
                            out=junk[:, W * t: W * (t + 1)],
                            data0=ptA[:, 512 * j: 512 * j + W],
                            data1=c2[:, W * j: W * j + W],
                            initial=INF, op0=ALU.min, op1=ALU.min)

                # ---------------- batch tails ----------------
                # contact: mind = junk[:, W-1::W]; contacts = exp(-100*mind)
                cont = sbt.tile([128, NT], f32, tag="cont")
                nc.scalar.activation(
                    cont[:],
                    junk[:].rearrange("p (t w) -> p t w", w=W)[:, :, W - 1],
                    AF.Exp, scale=-100.0)
                cdiff = sbt.tile([128, NT], f32, tag="cdiff")
                nc.gpsimd.tensor_tensor(cdiff[:], cont[:], hc[:],
                                        op=ALU.subtract)
                csq = sbt.tile([128, NT], bf16, tag="csq")
                nc.gpsimd.tensor_tensor(csq[:], cdiff[:], cdiff[:],
                                        op=ALU.mult)
                pcs = psS.tile([NT, 512], f32, tag="pss")
                nc.tensor.matmul(pcs[0:1, 0:NT], ones128b, csq[:],
                                 start=True, stop=True)
                nc.scalar.activation(acc[:, 256 + 128 * b: 256 + 128 * b + NT],
                                     pcs[0:1, 0:NT], AF.Copy)

                # choir: dsel = sqrt(max(d2selT, eps)); err = dsel - adT
                dselc = sbt.tile([NT, 128], f32, tag="dselc")
                nc.vector.tensor_scalar_max(dselc[:], d2selT[:, 0:128], 1.0e-12)
                dsel = sbt.tile([NT, 128], f32, tag="dsel")
                nc.scalar.activation(dsel[:], dselc[:], AF.Sqrt)
                ddiff = sbt.tile([NT, 128], f32, tag="ddiff")
                nc.gpsimd.tensor_tensor(ddiff[:], dsel[:], adT[:],
                                        op=ALU.subtract)
                dsq = sbt.tile([NT, 128], bf16, tag="dsq")
                nc.gpsimd.tensor_tensor(dsq[:], ddiff[:], ddiff[:],
                                        op=ALU.mult)
                pds = psS.tile([NT, 512], f32, tag="pss")
                nc.tensor.matmul(pds[0:1, 0:128], ones32b, dsq[:],
                                 start=True, stop=True)
                nc.scalar.activation(acc[:, 128 * b: 128 * b + 128],
                                     pds[0:1, 0:128], AF.Copy)

            # ---------------- final reduction ----------------
            res = sb.tile([1, 2], f32, tag="res")
            nc.vector.tensor_reduce(
                res[:, 0:1],
                acc[:].rearrange("p (j c) -> p j c", j=4)[:, 0:2, :],
                axis=AX.XY, op=ALU.add)
            nc.vector.tensor_reduce(
                res[:, 1:2],
                acc[:].rearrange("p (j c) -> p j c", j=4)[:, 2:4, :],
                axis=AX.XY, op=ALU.add)
            nc.sync.dma_start(out_d[:], res[:])

    nc.compile()
    return nc


def _get_program():
    if "nc" not in _CACHE:
        _CACHE["nc"] = _build_program()
    return _CACHE["nc"]


def _host_pack(verts, anchors, choir, hand_contacts, bps_mean, s, basis):
    """Build per-core input maps. p = 32q + tau (partition q, tile tau)."""
    inv_s = np.float32(1.0) / s
    inv_s2 = inv_s * inv_s

    # u = basis + delta  (B, P, 3); |u|^2
    delta = choir[:, :, 1:4]
    u = basis[None, :, :] + delta                      # (B, P, 3)
    usq = np.einsum('bpd,bpd->bp', u, u)               # (B, P)

    # selected anchor per point (host gather)
    idx = choir[:, :, 5].astype(np.int64)              # (B, P)
    asel = np.take_along_axis(anchors, idx[:, :, None], axis=1)  # (B, P, 3)

    # pack [10, NPAIR*128] tile-pair-major: row block 5j+d, col 128*kp+q
    # tile tau = 2*kp+j holds points p = 32q+tau
    def packT(vec3, sq):
        # vec3: (P, 3), sq: (P,) -> [10, NPAIR*128]
        outm = np.empty((10, NPAIR * 128), np.float32)
        # p = 32q + tau; tau = 2kp+j
        v = vec3.reshape(128, NT, 3)                   # [q, tau, d]
        q = sq.reshape(128, NT)
        for j in range(2):
            vj = v[:, j::2, :]                         # [q, kp, d]
            qj = q[:, j::2]
            blk = np.concatenate(
                [np.transpose(vj, (2, 1, 0)),          # [3, kp, q]
                 qj.T[None, :, :],                     # [1, kp, q]
                 np.ones((1, NPAIR, 128), np.float32)], axis=0)
            outm[5 * j: 5 * j + 5] = blk.reshape(5, NPAIR * 128)
        return outm

    B_, NT_, NP_ = B, NT, NPAIR
    uT = np.empty((B_, 10, NP_ * 128), np.float32)
    wsT = np.empty((B_, 10, NP_ * 128), np.float32)
    rhs = np.zeros((B_, 10, 2 * V), np.float32)
    hc = hand_contacts.reshape(B_, 128, NT_)
    adT = np.ascontiguousarray(
        choir[:, :, 4].reshape(B_, 128, NT_).transpose(0, 2, 1))

    for bb in range(B_):
        m = bps_mean[bb].reshape(3)
        uT[bb] = packT(u[bb], usq[bb])
        # wsel rows: [-2wx/s, -2wy/s, -2wz/s, 1/s^2, |w|^2], w = asel - m
        wa = asel[bb] - m[None, :]                     # (P, 3)
        wrows = np.concatenate(
            [(-2.0 * inv_s) * wa,
             np.full((P, 1), inv_s2, np.float32),
             np.einsum('pd,pd->p', wa, wa)[:, None]], axis=1)  # (P, 5)
        # reuse packT shape logic: treat rows 0:3 as vec3, 3 as sq, 4 as ones
        wv = wrows[:, 0:3]
        wq = wrows[:, 3]
        wo = wrows[:, 4]
        v = wv.reshape(128, NT_, 3)
        qq = wq.reshape(128, NT_)
        oo = wo.reshape(128, NT_)
        for j in range(2):
            blk = np.concatenate(
                [np.transpose(v[:, j::2, :], (2, 1, 0)),
                 qq[:, j::2].T[None, :, :],
                 oo[:, j::2].T[None, :, :]], axis=0)
            wsT[bb, 5 * j: 5 * j + 5] = blk.reshape(5, NP_ * 128)
        # rhs vert variants: w = vert - m
        wv778 = verts[bb] - m[None, :]                 # (V, 3)
        vr = np.concatenate(
            [(-2.0 * inv_s) * wv778,
             np.full((V, 1), inv_s2, np.float32),
             np.einsum('vd,vd->v', wv778, wv778)[:, None]], axis=1)  # (V,5)
        rhs[bb, 0:5, 0:V] = vr.T
        rhs[bb, 5:10, V:2 * V] = vr.T

    in_maps = []
    for c in range(NCORES):
        lo = BPC * c
        in_maps.append({
            "uT": uT[lo:lo + BPC],
            "ws": wsT[lo:lo + BPC],
            "rhs": rhs[lo:lo + BPC],
            "hc": np.ascontiguousarray(hc[lo:lo + BPC]),
            "adT": np.ascontiguousarray(adT[lo:lo + BPC]),
        })
    return in_maps


def kernel(verts, anchors, choir, hand_contacts, bps_mean, bps_scalar,
           bps_basis, _trace=False):
    from concourse.bass_utils import run_bass_kernel_spmd

    verts = np.ascontiguousarray(np.asarray(verts, np.float32))
    anchors = np.ascontiguousarray(np.asarray(anchors, np.float32))
    choir = np.ascontiguousarray(np.asarray(choir, np.float32))
    hand_contacts = np.ascontiguousarray(np.asarray(hand_contacts, np.float32))
    bps_mean = np.ascontiguousarray(np.asarray(bps_mean, np.float32))
    s = np.float32(np.asarray(bps_scalar).reshape(()))
    basis = np.ascontiguousarray(np.asarray(bps_basis, np.float32))

    nc = _get_program()
    in_maps = _host_pack(verts, anchors, choir, hand_contacts, bps_mean,
                         s, basis)
    res = run_bass_kernel_spmd(nc, in_maps, list(range(NCORES)), trace=_trace)
    parts = np.stack([np.asarray(r["partials"], np.float64).reshape(2)
                      for r in res.results])
    choir_loss = parts[:, 0].sum() / (B * P)
    contact_loss = parts[:, 1].sum() / (B * P)
    out = (np.float32(choir_loss), np.float32(contact_loss))
    if _trace:
        return out, res
    return out


# revision 13
# speedup vs baseline: 1.2255x; 1.2255x over previous
"""DualHOILoss Trainium2 kernel (8 NeuronCores, pure data parallel over batch).

Math (per batch b, point p, object o):
    u = basis_p + delta_p,  w_o = o - m_b
    d2[p,o] = |u/s - w|^2 = u.(-2w/s) + |u|^2/s^2 + |w|^2
computed as ONE f32r matmul with K=5 rows: lhsT rows [ux,uy,uz,|u|^2,1],
rhs rows [-2wx/s,-2wy/s,-2wz/s,1/s^2,|w|^2] -> PSUM holds the COMPLETE d2
(no per-point correction). Tiles processed in pairs sharing one K=10 lhsT
slice (rows 0:5 even tile, 5:10 odd tile; rhs has two zero-padded variants).

Per pair: PE writes 388 vert cols/tile into psB (2 banks, emitted first);
ACT drains the psB pair -> SBUF c2 (INF-padded to 390); PE writes the other
390 cols/tile into per-tile psA banks; DVE tensor_tensor_scan (min,min)
folds psA against c2 into junk. Pipeline depths: psA 3 tiles, psB 2 pairs,
c2 2 pairs -> DVE stays the only bottleneck. Choir path: host gathers the
selected anchor per point into ws rows (same 5-row semantics); Pool
multiplies uT*ws (bf16); per-pair selector matmuls accumulate d2sel
[NT,128] into one PSUM bank per batch region.

All tails run once at the end (both batches) so the ACT function table
switches Copy/Exp -> Sqrt exactly once. Host prep is layout-only packing
(transposes, anchor gather, |u|^2): all O(B*P*V) work is on device.
lhsT/rhs arrive as f32 and are rounded to f32r by chunked Pool copies
(birverifier requires a non-DMA last writer for f32r matmul operands).
Point tiling keeps the SBUF-natural map p = 32q + tau.
"""

import numpy as np

B, P, A, V = 16, 4096, 32, 778
NCORES = 8
BPC = B // NCORES      # batches per core
NT = P // 128          # 32 point tiles per batch
NPAIR = NT // 2        # 16 tile pairs per batch
N1 = 390               # vert cols scanned straight from PSUM (per tile)
N2 = V - N1            # vert cols drained to SBUF (388)
W = N1                 # scan width (c2 is INF-padded N2 -> W)
INF = 3.0e38
CHUNK = 4              # pair-chunks for f32r rounding copies

_CACHE = {}


def _build_program():
    import concourse.bacc as bacc
    import concourse.mybir as mybir
    from concourse import tile

    f32 = mybir.dt.float32
    f32r = mybir.dt.float32r
    bf16 = mybir.dt.bfloat16
    AF = mybir.ActivationFunctionType
    ALU = mybir.AluOpType
    AX = mybir.AxisListType

    nc = bacc.Bacc(None, target_bir_lowering=False)

    uT_d = nc.dram_tensor("uT", [BPC, 10, NPAIR * 128], f32, kind="ExternalInput")
    ws_d = nc.dram_tensor("ws", [BPC, 10, NPAIR * 128], f32, kind="ExternalInput")
    rhs_d = nc.dram_tensor("rhs", [BPC, 10, 2 * V], f32, kind="ExternalInput")
    hc_d = nc.dram_tensor("hc", [BPC, 128, NT], f32, kind="ExternalInput")
    adT_d = nc.dram_tensor("adT", [BPC, NT, 128], f32, kind="ExternalInput")
    out_d = nc.dram_tensor("partials", [1, 2], f32, kind="ExternalOutput")

    # per-pair selector matrices: selmats[k, 32*kp + tau] = 1 iff
    # (k<5 and tau==2kp) or (k>=5 and tau==2kp+1)
    selmats = np.zeros((10, 32 * NPAIR), np.float32)
    for kp in range(NPAIR):
        selmats[0:5, 32 * kp + 2 * kp] = 1.0
        selmats[5:10, 32 * kp + 2 * kp + 1] = 1.0
    selm_d = nc.inline_tensor(selmats, "selm")
    cst = np.zeros((128, 4), np.float32)
    cst[:, 0] = 1.0
    cst_d = nc.inline_tensor(cst, "cst")

    with tile.TileContext(nc) as tc:
        with (
            tc.tile_pool(name="sb", bufs=1) as sb,            # persistent
            tc.tile_pool(name="sbb", bufs=2) as sbb,          # per-batch
            tc.tile_pool(name="sbj", bufs=2) as sbj,          # junk per-batch
            tc.tile_pool(name="sbt", bufs=4) as sbt,          # small tail bufs
            tc.tile_pool(name="psA", bufs=3, space="PSUM") as psA,   # 3 banks
            tc.tile_pool(name="psB", bufs=2, space="PSUM") as psB,   # 4 banks
            tc.tile_pool(name="psS", bufs=1, space="PSUM") as psS,   # 1 bank
        ):
            # constants (bf16 conversions on Pool to keep DVE clean)
            cstf = sb.tile([128, 4], f32, tag="cstf")
            nc.sync.dma_start(cstf[:], cst_d[:])
            cstb = sb.tile([128, 4], bf16, tag="cstb")
            nc.gpsimd.tensor_copy(cstb[:], cstf[:])
            ones128b = cstb[:, 0:1]
            ones32b = cstb[0:NT, 0:1]
            selmf = sb.tile([10, 32 * NPAIR], f32, tag="selmf")
            nc.sync.dma_start(selmf[:], selm_d[:])
            selmb = sb.tile([10, 32 * NPAIR], bf16, tag="selmb")
            nc.gpsimd.tensor_copy(selmb[:], selmf[:])

            # c2 drain buffers with INF pads at cols N2:W per half
            c2bufs = []
            for i in range(2):
                c2 = sb.tile([128, 2 * W], f32, tag=f"c2_{i}")
                nc.gpsimd.memset(c2[:, N2:W], INF)
                nc.gpsimd.memset(c2[:, W + N2:], INF)
                c2bufs.append(c2)

            # d2sel accumulator: batch b in cols 256*b : 256*b+128
            d2selT = psS.tile([NT, 512], f32, tag="pss")

            junks = []
            uTs, hcs, adTs = [], [], []

            for b in range(BPC):
                # ---------------- per-batch loads + f32r rounding ----------
                uT_s = sbb.tile([10, NPAIR * 128], f32, tag="uT_s")
                nc.sync.dma_start(uT_s[:], uT_d[b])
                rhs_s = sbb.tile([10, 2 * V], f32, tag="rhs_s")
                nc.sync.dma_start(rhs_s[:], rhs_d[b])
                ws = sbb.tile([10, NPAIR * 128], f32, tag="ws")
                nc.sync.dma_start(ws[:], ws_d[b])
                hc = sbb.tile([128, NT], f32, tag="hc")
                nc.sync.dma_start(hc[:], hc_d[b])
                adT = sbb.tile([NT, 128], f32, tag="adT")
                nc.sync.dma_start(adT[:], adT_d[b])
                hcs.append(hc)
                adTs.append(adT)

                uT = sbb.tile([10, NPAIR * 128], f32r, tag="uT")
                rhs = sbb.tile([10, 2 * V], f32r, tag="rhs")
                prod = sbb.tile([10, NPAIR * 128], bf16, tag="prod")
                uTs.append(uT)
                CW = NPAIR * 128 // CHUNK
                # first uT chunk + rhs first so pair 0 unblocks early
                nc.gpsimd.tensor_copy(uT[:, 0:CW], uT_s[:, 0:CW])
                nc.gpsimd.tensor_copy(rhs[:, 0:V], rhs_s[:, 0:V])
                nc.gpsimd.tensor_copy(rhs[:, V:2 * V], rhs_s[:, V:2 * V])
                for cch in range(1, CHUNK):
                    nc.gpsimd.tensor_copy(
                        uT[:, CW * cch: CW * (cch + 1)],
                        uT_s[:, CW * cch: CW * (cch + 1)])
                for cch in range(CHUNK):
                    nc.gpsimd.tensor_tensor(
                        prod[:, CW * cch: CW * (cch + 1)],
                        uT[:, CW * cch: CW * (cch + 1)].bitcast(f32),
                        ws[:, CW * cch: CW * (cch + 1)], op=ALU.mult)

                junk = sbj.tile([128, W * NT], f32, tag="junk")
                junks.append(junk)

                # ---------------- per-pair main loop ----------------
                for kp in range(NPAIR):
                    lhsT = uT[:, 128 * kp: 128 * (kp + 1)]
                    ptB = psB.tile([128, 1024], f32, tag="ptB")
                    c2 = c2bufs[kp % 2]
                    for j in range(2):
                        rv = rhs[:, V * j: V * j + V]
                        nc.tensor.matmul(ptB[:, 512 * j: 512 * j + N2],
                                         lhsT, rv[:, N1:V],
                                         start=True, stop=True)
                    nc.scalar.activation(
                        c2bufs[kp % 2][:]
                            .rearrange("p (j w) -> p j w", j=2)[:, :, 0:N2],
                        ptB[:].rearrange("p (j w) -> p j w", j=2)[:, :, 0:N2],
                        AF.Copy)
                    for j in range(2):
                        t = 2 * kp + j
                        rv = rhs[:, V * j: V * j + V]
                        ptA = psA.tile([128, 512], f32, tag="ptA")
                        nc.tensor.matmul(ptA[:, 0:N1], lhsT, rv[:, 0:N1],
                                         start=True, stop=True)
                        nc.vector.tensor_tensor_scan(
                            out=junk[:, W * t: W * (t + 1)],
                            data0=ptA[:, 0:N1],
                            data1=c2[:, W * j: W * j + W],
                            initial=INF, op0=ALU.min, op1=ALU.min)
                    nc.tensor.matmul(d2selT[:, 256 * b: 256 * b + 128],
                                     selmb[:, 32 * kp: 32 * (kp + 1)],
                                     prod[:, 128 * kp: 128 * (kp + 1)],
                                     start=(kp == 0), stop=(kp == NPAIR - 1))

            # ---------------- tails (both batches; 1 act-table switch) -----
            acc = sb.tile([1, 4 * 128], f32, tag="acc")
            nc.gpsimd.memset(acc[:], 0.0)

            conts, dsels = [], []
            for b in range(BPC):
                cont = sbt.tile([128, NT], f32, tag="cont")
                nc.scalar.activation(
                    cont[:],
                    junks[b][:].rearrange("p (t w) -> p t w", w=W)[:, :, W - 1],
                    AF.Exp, scale=-100.0)
                conts.append(cont)
            dselc = sbt.tile([NT, 256], f32, tag="dselc")
            for b in range(BPC):
                nc.vector.tensor_scalar_max(
                    dselc[:, 128 * b: 128 * b + 128],
                    d2selT[:, 256 * b: 256 * b + 128], 1.0e-12)
            dsel = sbt.tile([NT, 256], f32, tag="dsel")
            nc.scalar.activation(dsel[:], dselc[:], AF.Sqrt)

            for b in range(BPC):
                cdiff = sbt.tile([128, NT], f32, tag="cdiff")
                nc.gpsimd.tensor_tensor(cdiff[:], conts[b][:], hcs[b][:],
                                        op=ALU.subtract)
                csq = sbt.tile([128, NT], bf16, tag="csq")
                nc.gpsimd.tensor_tensor(csq[:], cdiff[:], cdiff[:],
                                        op=ALU.mult)
                ddiff = sbt.tile([NT, 128], f32, tag="ddiff")
                nc.gpsimd.tensor_tensor(ddiff[:],
                                        dsel[:, 128 * b: 128 * b + 128],
                                        adTs[b][:], op=ALU.subtract)
                dsq = sbt.tile([NT, 128], bf16, tag="dsq")
                nc.gpsimd.tensor_tensor(dsq[:], ddiff[:], ddiff[:],
                                        op=ALU.mult)
                pcs = psS.tile([NT, 512], f32, tag="pss")
                nc.tensor.matmul(pcs[0:1, 0:NT], ones128b, csq[:],
                                 start=True, stop=True)
                nc.tensor.matmul(pcs[0:1, 128:256], ones32b, dsq[:],
                                 start=True, stop=True)
                nc.scalar.activation(acc[:, 256 + 128 * b: 256 + 128 * b + NT],
                                     pcs[0:1, 0:NT], AF.Copy)
                nc.scalar.activation(acc[:, 128 * b: 128 * b + 128],
                                     pcs[0:1, 128:256], AF.Copy)

            # ---------------- final reduction ----------------
            res = sb.tile([1, 2], f32, tag="res")
            nc.vector.tensor_reduce(
                res[:, 0:1],
                acc[:].rearrange("p (j c) -> p j c", j=4)[:, 0:2, :],
                axis=AX.XY, op=ALU.add)
            nc.vector.tensor_reduce(
                res[:, 1:2],
                acc[:].rearrange("p (j c) -> p j c", j=4)[:, 2:4, :],
                axis=AX.XY, op=ALU.add)
            nc.sync.dma_start(out_d[:], res[:])

    nc.compile()
    return nc


def _get_program():
    if "nc" not in _CACHE:
        _CACHE["nc"] = _build_program()
    return _CACHE["nc"]


def _host_pack(verts, anchors, choir, hand_contacts, bps_mean, s, basis):
    """Build per-core input maps. p = 32q + tau (partition q, tile tau)."""
    inv_s = np.float32(1.0) / s
    inv_s2 = inv_s * inv_s

    delta = choir[:, :, 1:4]
    u = basis[None, :, :] + delta                      # (B, P, 3)
    usq = np.einsum('bpd,bpd->bp', u, u)               # (B, P)

    idx = choir[:, :, 5].astype(np.int64)              # (B, P)
    asel = np.take_along_axis(anchors, idx[:, :, None], axis=1)  # (B, P, 3)

    def packT(vec3, sq, last):
        # vec3 (P,3), sq (P,), last (P,) -> [10, NPAIR*128]
        outm = np.empty((10, NPAIR * 128), np.float32)
        v = vec3.reshape(128, NT, 3)
        q = sq.reshape(128, NT)
        o = last.reshape(128, NT)
        for j in range(2):
            blk = np.concatenate(
                [np.transpose(v[:, j::2, :], (2, 1, 0)),
                 q[:, j::2].T[None, :, :],
                 o[:, j::2].T[None, :, :]], axis=0)
            outm[5 * j: 5 * j + 5] = blk.reshape(5, NPAIR * 128)
        return outm

    ones = np.ones(P, np.float32)
    uT = np.empty((B, 10, NPAIR * 128), np.float32)
    wsT = np.empty((B, 10, NPAIR * 128), np.float32)
    rhs = np.zeros((B, 10, 2 * V), np.float32)
    hc = hand_contacts.reshape(B, 128, NT)
    adT = np.ascontiguousarray(
        choir[:, :, 4].reshape(B, 128, NT).transpose(0, 2, 1))

    for bb in range(B):
        m = bps_mean[bb].reshape(3)
        uT[bb] = packT(u[bb], usq[bb], ones)
        wa = asel[bb] - m[None, :]
        wsT[bb] = packT((-2.0 * inv_s) * wa,
                        np.full(P, inv_s2, np.float32),
                        np.einsum('pd,pd->p', wa, wa))
        wv = verts[bb] - m[None, :]                    # (V, 3)
        vr = np.concatenate(
            [(-2.0 * inv_s) * wv,
             np.full((V, 1), inv_s2, np.float32),
             np.einsum('vd,vd->v', wv, wv)[:, None]], axis=1)  # (V, 5)
        rhs[bb, 0:5, 0:V] = vr.T
        rhs[bb, 5:10, V:2 * V] = vr.T

    in_maps = []
    for c in range(NCORES):
        lo = BPC * c
        in_maps.append({
            "uT": uT[lo:lo + BPC],
            "ws": wsT[lo:lo + BPC],
            "rhs": rhs[lo:lo + BPC],
            "hc": np.ascontiguousarray(hc[lo:lo + BPC]),
            "adT": np.ascontiguousarray(adT[lo:lo + BPC]),
        })
    return in_maps


def kernel(verts, anchors, choir, hand_contacts, bps_mean, bps_scalar,
           bps_basis, _trace=False):
    from concourse.bass_utils import run_bass_kernel_spmd

    verts = np.ascontiguousarray(np.asarray(verts, np.float32))
    anchors = np.ascontiguousarray(np.asarray(anchors, np.float32))
    choir = np.ascontiguousarray(np.asarray(choir, np.float32))
    hand_contacts = np.ascontiguousarray(np.asarray(hand_contacts, np.float32))
    bps_mean = np.ascontiguousarray(np.asarray(bps_mean, np.float32))
    s = np.float32(np.asarray(bps_scalar).reshape(()))
    basis = np.ascontiguousarray(np.asarray(bps_basis, np.float32))

    nc = _get_program()
    in_maps = _host_pack(verts, anchors, choir, hand_contacts, bps_mean,
                         s, basis)
    res = run_bass_kernel_spmd(nc, in_maps, list(range(NCORES)), trace=_trace)
    parts = np.stack([np.asarray(r["partials"], np.float64).reshape(2)
                      for r in res.results])
    choir_loss = parts[:, 0].sum() / (B * P)
    contact_loss = parts[:, 1].sum() / (B * P)
    out = (np.float32(choir_loss), np.float32(contact_loss))
    if _trace:
        return out, res
    return out
